# revision 7
# baseline (speedup 1.0000x reference)
"""Trainium2 Bass kernel for DifferentiableWeightedRadialFrequencyLoss.

Math:
  loss = sum_b w_b * sum_{n,c} sum_{(u,v) in band b} |FFT2(pred-gt)|^2 / (N*C*H*W)
       = sum_{n,c,u,v} Wmap[u,v] * |FFT2(err_nc)[u,v]|^2 / size
where Wmap = sum_b w_b * mask_b (bands are disjoint), evaluated in unshifted
frequency coordinates (ifftshift of the precomputed radial map).

Device algorithm (per core, 12 images = 6 pairs):
  - pack two real images per complex FFT: Z = E1 + i*E2.  Since Wmap is
    symmetric under (u,v) -> (-u,-v), the cross terms cancel exactly and
    sum(W*|F_z|^2) = sum(W*(|F_1|^2 + |F_2|^2)).
  - FFT2 via two matmul stages with the symmetric DFT matrix D (ortho norm):
      out1 = Z^T @ D          (stage 1; lhsT = Z chunks)
      F^T  = D @ out1         (stage 2; lhsT = D chunks)
  - weighted power: G = sqrtW^T ⊙ F^T ; acc += row-sum(G^2)  (DVE fused ops)
Host: shard batch across 8 cores, sum partial accumulators, divide by size.
"""

import numpy as np
import ml_dtypes

import concourse.bass as bass
import concourse.bacc as bacc
import concourse.tile as tile
from concourse import mybir
from concourse.bass_utils import run_bass_kernel_spmd

N_CORES = 8
N, C, H = 32, 3, 512
NUM_BANDS = 16
IMGS_PER_CORE = (N // N_CORES) * C          # 12
PAIRS = IMGS_PER_CORE // 2                  # 6
F32 = mybir.dt.float32
BF16 = mybir.dt.bfloat16

# exposed for test.py introspection (exec time / trace)
last_results = None
last_nc = None
last_in_maps = None


def _build_nc():
    nc = bacc.Bacc("TRN2", target_bir_lowering=False, debug=False,
                   num_devices=N_CORES)
    pred = nc.dram_tensor("pred", [IMGS_PER_CORE, H, H], F32, kind="ExternalInput")
    gt = nc.dram_tensor("gt", [IMGS_PER_CORE, H, H], F32, kind="ExternalInput")
    dr = nc.dram_tensor("dr", [H, H], BF16, kind="ExternalInput")
    di = nc.dram_tensor("di", [H, H], BF16, kind="ExternalInput")
    dn = nc.dram_tensor("dn", [H, H], BF16, kind="ExternalInput")  # -Di
    swt = nc.dram_tensor("swt", [H, H], F32, kind="ExternalInput")  # sqrt(W)^T
    out = nc.dram_tensor("out", [128, 8 * PAIRS], F32, kind="ExternalOutput")

    def r4(ap):  # [512, 512] dram view -> [128 part, 4 chunks, 512]
        return ap.rearrange("(c p) w -> p c w", p=128)

    with tile.TileContext(nc) as tc:
        with (
            tc.tile_pool(name="consts", bufs=1) as consts,
            tc.tile_pool(name="loads", bufs=2) as loads,
            tc.tile_pool(name="zpool", bufs=2) as zpool,
            tc.tile_pool(name="o1pool", bufs=2) as o1pool,
            tc.tile_pool(name="spool", bufs=4) as spool,
            tc.tile_pool(name="ps1", bufs=2, space="PSUM") as ps1,
            tc.tile_pool(name="ps2", bufs=2, space="PSUM") as ps2,
        ):
            dr_sb = consts.tile([128, 4, H], BF16)
            di_sb = consts.tile([128, 4, H], BF16)
            dn_sb = consts.tile([128, 4, H], BF16)
            swt_sb = consts.tile([128, 4, H], F32)
            acc = consts.tile([128, 8 * PAIRS], F32)
            nc.sync.dma_start(out=dr_sb[:], in_=r4(dr.ap()))
            nc.sync.dma_start(out=di_sb[:], in_=r4(di.ap()))
            nc.sync.dma_start(out=dn_sb[:], in_=r4(dn.ap()))
            nc.sync.dma_start(out=swt_sb[:], in_=r4(swt.ap()))

            for pr in range(PAIRS):
                i1, i2 = 2 * pr, 2 * pr + 1
                p1t = loads.tile([128, 4, H], F32, tag="p1t")
                g1t = loads.tile([128, 4, H], F32, tag="g1t")
                p2t = loads.tile([128, 4, H], F32, tag="p2t")
                g2t = loads.tile([128, 4, H], F32, tag="g2t")
                nc.sync.dma_start(out=p1t[:], in_=r4(pred.ap()[i1]))
                nc.sync.dma_start(out=g1t[:], in_=r4(gt.ap()[i1]))
                nc.sync.dma_start(out=p2t[:], in_=r4(pred.ap()[i2]))
                nc.sync.dma_start(out=g2t[:], in_=r4(gt.ap()[i2]))

                zr = zpool.tile([128, 4, H], BF16, tag="zr")
                zi = zpool.tile([128, 4, H], BF16, tag="zi")
                nc.vector.tensor_sub(zr[:], p1t[:], g1t[:])
                nc.vector.tensor_sub(zi[:], p2t[:], g2t[:])

                # stage 1: out1 = Z^T @ D  (out1[p,q], psum partition = p chunk)
                o1r = o1pool.tile([128, 4, H], BF16, tag="o1r")
                o1i = o1pool.tile([128, 4, H], BF16, tag="o1i")
                for m in range(4):
                    psr = ps1.tile([128, H], F32, tag="s1r")
                    psi = ps1.tile([128, H], F32, tag="s1i")
                    for k in range(4):
                        zrk = zr[:, k, m * 128:(m + 1) * 128]
                        nc.tensor.matmul(psr[:], zrk, dr_sb[:, k, :],
                                         start=(k == 0), stop=False)
                        nc.tensor.matmul(psi[:], zrk, di_sb[:, k, :],
                                         start=(k == 0), stop=False)
                    for k in range(4):
                        zik = zi[:, k, m * 128:(m + 1) * 128]
                        nc.tensor.matmul(psr[:], zik, dn_sb[:, k, :],
                                         start=False, stop=(k == 3))
                        nc.tensor.matmul(psi[:], zik, dr_sb[:, k, :],
                                         start=False, stop=(k == 3))
                    nc.scalar.copy(out=o1r[:, m, :], in_=psr[:])
                    nc.scalar.copy(out=o1i[:, m, :], in_=psi[:])

                # stage 2: F^T = D @ out1 ; stage 3: weighted power accumulate
                for v in range(4):
                    fr = ps2.tile([128, H], F32, tag="s2r")
                    fi = ps2.tile([128, H], F32, tag="s2i")
                    for p in range(4):
                        drp = dr_sb[:, p, v * 128:(v + 1) * 128]
                        nc.tensor.matmul(fr[:], drp, o1r[:, p, :],
                                         start=(p == 0), stop=False)
                        nc.tensor.matmul(fi[:], drp, o1i[:, p, :],
                                         start=(p == 0), stop=False)
                    for p in range(4):
                        dnp = dn_sb[:, p, v * 128:(v + 1) * 128]
                        dip = di_sb[:, p, v * 128:(v + 1) * 128]
                        nc.tensor.matmul(fr[:], dnp, o1i[:, p, :],
                                         start=False, stop=(p == 3))
                        nc.tensor.matmul(fi[:], dip, o1r[:, p, :],
                                         start=False, stop=(p == 3))
                    for ps, j in ((fr, 8 * pr + 2 * v), (fi, 8 * pr + 2 * v + 1)):
                        g = spool.tile([128, H], BF16, tag="g")
                        gs = spool.tile([128, H], BF16, tag="gs")
                        nc.vector.scalar_tensor_tensor(
                            out=g[:], in0=ps[:], scalar=0.0, in1=swt_sb[:, v, :],
                            op0=mybir.AluOpType.bypass, op1=mybir.AluOpType.mult)
                        nc.vector.scalar_tensor_tensor(
                            out=gs[:], in0=g[:], scalar=0.0, in1=g[:],
                            op0=mybir.AluOpType.bypass,
                            op1=mybir.AluOpType.mult,
                            accum_out=acc[:, j:j + 1])

            nc.sync.dma_start(out=out.ap(), in_=acc[:])

    nc.compile()
    return nc


def kernel(predictions, ground_truths, band_weights, band_masks):
    global last_results, last_nc, last_in_maps
    pred = np.ascontiguousarray(np.asarray(predictions, dtype=np.float32))
    gt = np.ascontiguousarray(np.asarray(ground_truths, dtype=np.float32))
    bw = np.asarray(band_weights, dtype=np.float64)
    bm = np.asarray(band_masks, dtype=np.float64)

    # host-side prep of tiny replicated constants
    wmap = np.einsum('b,bhw->hw', bw, bm)          # shifted coords
    wu = np.fft.ifftshift(wmap)                     # unshifted coords
    swt = np.ascontiguousarray(np.sqrt(wu).T.astype(np.float32))
    j = np.arange(H, dtype=np.float64)
    ang = 2.0 * np.pi * np.outer(j, j) / H
    scale = 1.0 / np.sqrt(H)
    drm = (np.cos(ang) * scale)
    dim = (-np.sin(ang) * scale)
    bf = ml_dtypes.bfloat16
    drb = np.ascontiguousarray(drm.astype(bf))
    dib = np.ascontiguousarray(dim.astype(bf))
    dnb = np.ascontiguousarray((-dim).astype(bf))

    pred_r = pred.reshape(N_CORES, IMGS_PER_CORE, H, H)
    gt_r = gt.reshape(N_CORES, IMGS_PER_CORE, H, H)
    in_maps = [
        {
            "pred": np.ascontiguousarray(pred_r[c]),
            "gt": np.ascontiguousarray(gt_r[c]),
            "dr": drb, "di": dib, "dn": dnb, "swt": swt,
        }
        for c in range(N_CORES)
    ]

    nc = _build_nc()
    last_nc, last_in_maps = nc, in_maps
    res = run_bass_kernel_spmd(nc, in_maps, core_ids=list(range(N_CORES)))
    last_results = res
    total = np.float64(0.0)
    for r in res.results:
        total += r["out"].astype(np.float64).sum()
    loss = total / float(N * C * H * H)
    return np.float32(loss)


# revision 14
# speedup vs baseline: 1.1202x; 1.1202x over previous
"""Trainium2 Bass kernel for DifferentiableWeightedRadialFrequencyLoss.

Math:
  loss = sum_{n,c,u,v} Wmap[u,v] * |FFT2(pred-gt)[u,v]|^2 / size
with Wmap = sum_b w_b * mask_b (bands disjoint), in unshifted (ifftshift)
frequency coordinates.

Device algorithm (per core, 12 images = 6 pairs):
  - pack two real images per complex FFT: Z = E1 + i*E2 (Wmap is symmetric
    under (u,v) -> (-u,-v), so cross terms cancel exactly).
  - FFT2 as two matmul stages with the symmetric ortho DFT matrix D:
      out1 = Z^T @ D      (stage 1)
      F^T  = D @ out1     (stage 2)
    each complex product via 3-multiplication Karatsuba:
      (A+iB)@(C+iD): m1=A@(C+D), m2=(A+B)@D, m3=(B-A)@C
                     real=m1-m2, imag=m1+m3
    with the constant-side combos (Dr+Di, Di-Dr) precomputed on host.
  - weighted power: P = Fr^2 + Fi^2 (ACT squares), acc += row-sum(P .* W^T)
    via DVE scalar_tensor_tensor accum_out.
Host: shard batch across 8 cores, sum partial accumulators, divide by size.
"""

import numpy as np
import ml_dtypes

import concourse.bass as bass
import concourse.bacc as bacc
import concourse.tile as tile
from concourse import mybir
from concourse.bass_utils import run_bass_kernel_spmd

N_CORES = 8
N, C, H = 32, 3, 512
NUM_BANDS = 16
IMGS_PER_CORE = (N // N_CORES) * C          # 12
PAIRS = IMGS_PER_CORE // 2                  # 6
F32 = mybir.dt.float32
BF16 = mybir.dt.bfloat16
ALU = mybir.AluOpType

# exposed for test.py introspection
last_results = None
last_nc = None
last_in_maps = None


def _build_nc(s2_3m=True, combo_eng="dve", xs_chunked=True,
              zbufs=3, o1bufs=3, ldbufs=2, tadd_eng="dve", spbufs=3):
    nc = bacc.Bacc("TRN2", target_bir_lowering=False, debug=False,
                   num_devices=N_CORES)
    pred = nc.dram_tensor("pred", [IMGS_PER_CORE, H, H], F32, kind="ExternalInput")
    gt = nc.dram_tensor("gt", [IMGS_PER_CORE, H, H], F32, kind="ExternalInput")
    d_r = nc.dram_tensor("d_r", [H, H], BF16, kind="ExternalInput")   # Dr
    d_i = nc.dram_tensor("d_i", [H, H], BF16, kind="ExternalInput")   # Di
    d_p = nc.dram_tensor("d_p", [H, H], BF16, kind="ExternalInput")   # Dr+Di
    d_m = nc.dram_tensor("d_m", [H, H], BF16, kind="ExternalInput")   # Di-Dr
    d_n = nc.dram_tensor("d_n", [H, H], BF16, kind="ExternalInput")   # -Di
    wt = nc.dram_tensor("wt", [H, H], BF16, kind="ExternalInput")     # W^T
    out = nc.dram_tensor("out", [128, 4 * PAIRS], F32, kind="ExternalOutput")

    def r4(ap):  # [512, 512] dram view -> [128 part, 4 chunks, 512]
        return ap.rearrange("(c p) w -> p c w", p=128)

    with tile.TileContext(nc) as tc:
        with (
            tc.tile_pool(name="consts", bufs=1) as consts,
            tc.tile_pool(name="loads", bufs=ldbufs) as loads,
            tc.tile_pool(name="zpool", bufs=zbufs) as zpool,
            tc.tile_pool(name="o1pool", bufs=o1bufs) as o1pool,
            tc.tile_pool(name="spool", bufs=spbufs) as spool,
            tc.tile_pool(name="ps1a", bufs=2, space="PSUM") as ps1a,
            tc.tile_pool(name="ps1bc", bufs=1, space="PSUM") as ps1bc,
            tc.tile_pool(name="ps2a", bufs=2, space="PSUM") as ps2a,
            tc.tile_pool(name="ps2bc", bufs=1, space="PSUM") as ps2bc,
        ):
            dr_sb = consts.tile([128, 4, H], BF16)
            di_sb = consts.tile([128, 4, H], BF16)
            dp_sb = consts.tile([128, 4, H], BF16)
            dm_sb = consts.tile([128, 4, H], BF16)
            dn_sb = consts.tile([128, 4, H], BF16)
            wt_sb = consts.tile([128, 4, H], BF16)
            acc = consts.tile([128, 4 * PAIRS], F32)
            nc.sync.dma_start(out=dr_sb[:], in_=r4(d_r.ap()))
            nc.sync.dma_start(out=di_sb[:], in_=r4(d_i.ap()))
            nc.sync.dma_start(out=dp_sb[:], in_=r4(d_p.ap()))
            nc.sync.dma_start(out=dm_sb[:], in_=r4(d_m.ap()))
            nc.sync.dma_start(out=dn_sb[:], in_=r4(d_n.ap()))
            nc.sync.dma_start(out=wt_sb[:], in_=r4(wt.ap()))

            for pr in range(PAIRS):
                i1, i2 = 2 * pr, 2 * pr + 1
                p1t = loads.tile([128, 4, H], F32, tag="p1t")
                g1t = loads.tile([128, 4, H], F32, tag="g1t")
                p2t = loads.tile([128, 4, H], F32, tag="p2t")
                g2t = loads.tile([128, 4, H], F32, tag="g2t")
                nc.sync.dma_start(out=p1t[:], in_=r4(pred.ap()[i1]))
                nc.sync.dma_start(out=g1t[:], in_=r4(gt.ap()[i1]))
                nc.sync.dma_start(out=p2t[:], in_=r4(pred.ap()[i2]))
                nc.sync.dma_start(out=g2t[:], in_=r4(gt.ap()[i2]))

                # data tiles: zr = E1, zi = E2, zs = zr+zi, zd = zi-zr
                zr = zpool.tile([128, 4, H], BF16, tag="zr")
                zi = zpool.tile([128, 4, H], BF16, tag="zi")
                zs = zpool.tile([128, 4, H], BF16, tag="zs")
                zd = zpool.tile([128, 4, H], BF16, tag="zd")
                ce = nc.gpsimd if combo_eng == "pool" else nc.vector
                nc.gpsimd.tensor_sub(zr[:], p1t[:], g1t[:])
                nc.gpsimd.tensor_sub(zi[:], p2t[:], g2t[:])
                ce.tensor_add(zs[:], zr[:], zi[:])
                ce.tensor_sub(zd[:], zi[:], zr[:])

                # stage 1: out1 = Z^T @ D via 3M
                o1r = o1pool.tile([128, 4, H], BF16, tag="o1r")
                o1i = o1pool.tile([128, 4, H], BF16, tag="o1i")
                for m in range(4):
                    sl = slice(m * 128, (m + 1) * 128)
                    pa = ps1a.tile([128, H], F32, tag="a")
                    pb = ps1bc.tile([128, H], F32, tag="b")
                    for k in range(4):
                        nc.tensor.matmul(pa[:], zr[:, k, sl], dp_sb[:, k, :],
                                         start=(k == 0), stop=(k == 3))
                        nc.tensor.matmul(pb[:], zs[:, k, sl], di_sb[:, k, :],
                                         start=(k == 0), stop=(k == 3))
                    pa_sb = spool.tile([128, H], F32, tag="pas")
                    nc.scalar.copy(pa_sb[:], pa[:])
                    nc.vector.tensor_sub(o1r[:, m, :], pa_sb[:], pb[:])
                    pc = ps1bc.tile([128, H], F32, tag="c")
                    for k in range(4):
                        nc.tensor.matmul(pc[:], zd[:, k, sl], dr_sb[:, k, :],
                                         start=(k == 0), stop=(k == 3))
                    nc.vector.tensor_add(o1i[:, m, :], pa_sb[:], pc[:])
                xs = o1pool.tile([128, 4, H], BF16, tag="xs")
                if s2_3m:
                    if xs_chunked:
                        for m in range(4):
                            ce.tensor_add(xs[:, m, :], o1r[:, m, :], o1i[:, m, :])
                    else:
                        ce.tensor_add(xs[:], o1r[:], o1i[:])

                # stage 2: F^T = D @ out1 via 3M ; weighted power accumulate
                for v in range(4):
                    sl = slice(v * 128, (v + 1) * 128)
                    if s2_3m:
                        pa = ps2a.tile([128, H], F32, tag="a2")
                        pb = ps2bc.tile([128, H], F32, tag="b2")
                        for p in range(4):
                            nc.tensor.matmul(pa[:], dr_sb[:, p, sl], xs[:, p, :],
                                             start=(p == 0), stop=(p == 3))
                            nc.tensor.matmul(pb[:], dp_sb[:, p, sl], o1i[:, p, :],
                                             start=(p == 0), stop=(p == 3))
                        pa2_sb = spool.tile([128, H], F32, tag="pas2")
                        nc.scalar.copy(pa2_sb[:], pa[:])
                        fr = spool.tile([128, H], BF16, tag="fr")
                        nc.vector.tensor_sub(fr[:], pa2_sb[:], pb[:])
                        pc = ps2bc.tile([128, H], F32, tag="c2")
                        for p in range(4):
                            nc.tensor.matmul(pc[:], dm_sb[:, p, sl], o1r[:, p, :],
                                             start=(p == 0), stop=(p == 3))
                        fi = spool.tile([128, H], BF16, tag="fi")
                        nc.vector.tensor_add(fi[:], pa2_sb[:], pc[:])
                        prt = spool.tile([128, H], BF16, tag="prt")
                        pit = spool.tile([128, H], BF16, tag="pit")
                        nc.scalar.square(prt[:], fr[:])
                        nc.scalar.square(pit[:], fi[:])
                    else:
                        pa = ps2a.tile([128, H], F32, tag="a2")
                        pb = ps2a.tile([128, H], F32, tag="b2")
                        for p in range(4):
                            drp = dr_sb[:, p, sl]
                            nc.tensor.matmul(pa[:], drp, o1r[:, p, :],
                                             start=(p == 0), stop=False)
                            nc.tensor.matmul(pb[:], drp, o1i[:, p, :],
                                             start=(p == 0), stop=False)
                        for p in range(4):
                            nc.tensor.matmul(pa[:], dn_sb[:, p, sl], o1i[:, p, :],
                                             start=False, stop=(p == 3))
                            nc.tensor.matmul(pb[:], di_sb[:, p, sl], o1r[:, p, :],
                                             start=False, stop=(p == 3))
                        prt = spool.tile([128, H], BF16, tag="prt")
                        pit = spool.tile([128, H], BF16, tag="pit")
                        nc.scalar.square(prt[:], pa[:])
                        nc.scalar.square(pit[:], pb[:])
                    t = spool.tile([128, H], BF16, tag="t")
                    te = nc.gpsimd if tadd_eng == "pool" else nc.vector
                    te.tensor_add(t[:], prt[:], pit[:])
                    gs = spool.tile([128, H], BF16, tag="gs")
                    nc.vector.scalar_tensor_tensor(
                        out=gs[:], in0=t[:], scalar=0.0, in1=wt_sb[:, v, :],
                        op0=ALU.bypass, op1=ALU.mult,
                        accum_out=acc[:, 4 * pr + v: 4 * pr + v + 1])

            nc.sync.dma_start(out=out.ap(), in_=acc[:])

    nc.compile()
    return nc


def kernel(predictions, ground_truths, band_weights, band_masks):
    global last_results, last_nc, last_in_maps
    pred = np.ascontiguousarray(np.asarray(predictions, dtype=np.float32))
    gt = np.ascontiguousarray(np.asarray(ground_truths, dtype=np.float32))
    bw = np.asarray(band_weights, dtype=np.float64)
    bm = np.asarray(band_masks, dtype=np.float64)

    # host-side prep of tiny replicated constants
    wmap = np.einsum('b,bhw->hw', bw, bm)          # shifted coords
    wu = np.fft.ifftshift(wmap)                     # unshifted coords
    bf = ml_dtypes.bfloat16
    wtb = np.ascontiguousarray(wu.T.astype(bf))
    j = np.arange(H, dtype=np.float64)
    ang = 2.0 * np.pi * np.outer(j, j) / H
    scale = 1.0 / np.sqrt(H)
    drm = np.cos(ang) * scale
    dim = -np.sin(ang) * scale
    drb = np.ascontiguousarray(drm.astype(bf))
    dib = np.ascontiguousarray(dim.astype(bf))
    dpb = np.ascontiguousarray((drm + dim).astype(bf))
    dmb = np.ascontiguousarray((dim - drm).astype(bf))
    dnb = np.ascontiguousarray((-dim).astype(bf))

    pred_r = pred.reshape(N_CORES, IMGS_PER_CORE, H, H)
    gt_r = gt.reshape(N_CORES, IMGS_PER_CORE, H, H)
    in_maps = [
        {
            "pred": np.ascontiguousarray(pred_r[c]),
            "gt": np.ascontiguousarray(gt_r[c]),
            "d_r": drb, "d_i": dib, "d_p": dpb, "d_m": dmb, "d_n": dnb,
            "wt": wtb,
        }
        for c in range(N_CORES)
    ]

    nc = _build_nc()
    last_nc, last_in_maps = nc, in_maps
    res = run_bass_kernel_spmd(nc, in_maps, core_ids=list(range(N_CORES)))
    last_results = res
    total = np.float64(0.0)
    for r in res.results:
        total += r["out"].astype(np.float64).sum()
    loss = total / float(N * C * H * H)
    return np.float32(loss)


# revision 17
# speedup vs baseline: 1.3568x; 1.2111x over previous
"""Trainium2 Bass kernel for DifferentiableWeightedRadialFrequencyLoss.

Math:
  loss = sum_{n,c,u,v} Wmap[u,v] * |FFT2(pred-gt)[u,v]|^2 / size
with Wmap = sum_b w_b * mask_b (bands disjoint), in unshifted (ifftshift)
frequency coordinates.

Device algorithm (per core, 12 images = 6 pairs):
  - pack two real images per complex FFT: Z = E1 + i*E2 (Wmap is symmetric
    under (u,v) -> (-u,-v), so cross terms cancel exactly).
  - FFT2 as two matmul stages with the symmetric ortho DFT matrix D:
      out1 = Z^T @ D      (stage 1)
      F^T  = D @ out1     (stage 2)
    each complex product via 3-multiplication Karatsuba:
      (A+iB)@(C+iD): m1=A@(C+D), m2=(A+B)@D, m3=(B-A)@C
                     real=m1-m2, imag=m1+m3
    with the constant-side combos (Dr+Di, Di-Dr) precomputed on host.
  - weighted power: P = Fr^2 + Fi^2 (ACT squares), acc += row-sum(P .* W^T)
    via DVE scalar_tensor_tensor accum_out.
Host: shard batch across 8 cores, sum partial accumulators, divide by size.
"""

import numpy as np
import ml_dtypes

import concourse.bass as bass
import concourse.bacc as bacc
import concourse.tile as tile
from concourse import mybir
from concourse.bass_utils import run_bass_kernel_spmd

N_CORES = 8
N, C, H = 32, 3, 512
NUM_BANDS = 16
IMGS_PER_CORE = (N // N_CORES) * C          # 12
PAIRS = IMGS_PER_CORE // 2                  # 6
F32 = mybir.dt.float32
BF16 = mybir.dt.bfloat16
ALU = mybir.AluOpType

# exposed for test.py introspection
last_results = None
last_nc = None
last_in_maps = None


def _build_nc(s2_3m=True, combo_eng="dve", xs_chunked=True,
              zbufs=3, o1bufs=2, ldbufs=2, tadd_eng="dve", spbufs=3):
    nc = bacc.Bacc("TRN2", target_bir_lowering=False, debug=False,
                   num_devices=N_CORES)
    pred = nc.dram_tensor("pred", [IMGS_PER_CORE, H, H], F32, kind="ExternalInput")
    gt = nc.dram_tensor("gt", [IMGS_PER_CORE, H, H], F32, kind="ExternalInput")
    d_r = nc.dram_tensor("d_r", [H, H], BF16, kind="ExternalInput")   # Dr
    d_i = nc.dram_tensor("d_i", [H, H], BF16, kind="ExternalInput")   # Di
    d_p = nc.dram_tensor("d_p", [H, H], BF16, kind="ExternalInput")   # Dr+Di
    d_m = nc.dram_tensor("d_m", [H, H], BF16, kind="ExternalInput")   # Di-Dr
    d_n = nc.dram_tensor("d_n", [H, H], BF16, kind="ExternalInput")   # -Di
    wt = nc.dram_tensor("wt", [H, H], BF16, kind="ExternalInput")     # W^T
    out = nc.dram_tensor("out", [128, PAIRS], F32, kind="ExternalOutput")

    def r4(ap):  # [512, 512] dram view -> [128 part, 4 chunks, 512]
        return ap.rearrange("(c p) w -> p c w", p=128)

    with tile.TileContext(nc) as tc:
        with (
            tc.tile_pool(name="consts", bufs=1) as consts,
            tc.tile_pool(name="loads", bufs=ldbufs) as loads,
            tc.tile_pool(name="zpool", bufs=zbufs) as zpool,
            tc.tile_pool(name="o1pool", bufs=o1bufs) as o1pool,
            tc.tile_pool(name="spool", bufs=spbufs) as spool,
            tc.tile_pool(name="bigsc", bufs=2) as bigsc,
            tc.tile_pool(name="ps1a", bufs=2, space="PSUM") as ps1a,
            tc.tile_pool(name="ps1bc", bufs=1, space="PSUM") as ps1bc,
            tc.tile_pool(name="ps2a", bufs=2, space="PSUM") as ps2a,
            tc.tile_pool(name="ps2bc", bufs=1, space="PSUM") as ps2bc,
        ):
            dr_sb = consts.tile([128, 4, H], BF16)
            di_sb = consts.tile([128, 4, H], BF16)
            dp_sb = consts.tile([128, 4, H], BF16)
            dm_sb = consts.tile([128, 4, H], BF16)
            dn_sb = None if s2_3m else consts.tile([128, 4, H], BF16)
            wt_sb = consts.tile([128, 4, H], BF16)
            acc = consts.tile([128, PAIRS], F32)
            nc.sync.dma_start(out=dr_sb[:], in_=r4(d_r.ap()))
            nc.sync.dma_start(out=di_sb[:], in_=r4(d_i.ap()))
            nc.sync.dma_start(out=dp_sb[:], in_=r4(d_p.ap()))
            nc.sync.dma_start(out=dm_sb[:], in_=r4(d_m.ap()))
            if dn_sb is not None:
                nc.sync.dma_start(out=dn_sb[:], in_=r4(d_n.ap()))
            nc.sync.dma_start(out=wt_sb[:], in_=r4(wt.ap()))

            for pr in range(PAIRS):
                i1, i2 = 2 * pr, 2 * pr + 1
                p1t = loads.tile([128, 4, H], F32, tag="p1t")
                g1t = loads.tile([128, 4, H], F32, tag="g1t")
                p2t = loads.tile([128, 4, H], F32, tag="p2t")
                g2t = loads.tile([128, 4, H], F32, tag="g2t")
                nc.sync.dma_start(out=p1t[:], in_=r4(pred.ap()[i1]))
                nc.sync.dma_start(out=g1t[:], in_=r4(gt.ap()[i1]))
                nc.sync.dma_start(out=p2t[:], in_=r4(pred.ap()[i2]))
                nc.sync.dma_start(out=g2t[:], in_=r4(gt.ap()[i2]))

                # data tiles: zr = E1, zi = E2, zs = zr+zi, zd = zi-zr
                zr = zpool.tile([128, 4, H], BF16, tag="zr")
                zi = zpool.tile([128, 4, H], BF16, tag="zi")
                zs = zpool.tile([128, 4, H], BF16, tag="zs")
                zd = zpool.tile([128, 4, H], BF16, tag="zd")
                ce = nc.gpsimd if combo_eng == "pool" else nc.vector
                nc.gpsimd.tensor_sub(zr[:], p1t[:], g1t[:])
                nc.gpsimd.tensor_sub(zi[:], p2t[:], g2t[:])
                ce.tensor_add(zs[:], zr[:], zi[:])
                ce.tensor_sub(zd[:], zi[:], zr[:])

                # stage 1: out1 = Z^T @ D via 3M
                o1r = o1pool.tile([128, 4, H], BF16, tag="o1r")
                o1i = o1pool.tile([128, 4, H], BF16, tag="o1i")
                for m in range(4):
                    sl = slice(m * 128, (m + 1) * 128)
                    pa = ps1a.tile([128, H], F32, tag="a")
                    pb = ps1bc.tile([128, H], F32, tag="b")
                    for k in range(4):
                        nc.tensor.matmul(pa[:], zr[:, k, sl], dp_sb[:, k, :],
                                         start=(k == 0), stop=(k == 3))
                        nc.tensor.matmul(pb[:], zs[:, k, sl], di_sb[:, k, :],
                                         start=(k == 0), stop=(k == 3))
                    pa_sb = spool.tile([128, H], F32, tag="pas")
                    nc.scalar.copy(pa_sb[:], pa[:])
                    nc.vector.tensor_sub(o1r[:, m, :], pa_sb[:], pb[:])
                    pc = ps1bc.tile([128, H], F32, tag="c")
                    for k in range(4):
                        nc.tensor.matmul(pc[:], zd[:, k, sl], dr_sb[:, k, :],
                                         start=(k == 0), stop=(k == 3))
                    nc.vector.tensor_add(o1i[:, m, :], pa_sb[:], pc[:])
                xs = o1pool.tile([128, 4, H], BF16, tag="xs")
                if s2_3m:
                    if xs_chunked:
                        for m in range(4):
                            ce.tensor_add(xs[:, m, :], o1r[:, m, :], o1i[:, m, :])
                    else:
                        ce.tensor_add(xs[:], o1r[:], o1i[:])

                # stage 2: F^T = D @ out1 via 3M ; weighted power accumulate
                prt = bigsc.tile([128, 4, H], BF16, tag="prt")
                pit = bigsc.tile([128, 4, H], BF16, tag="pit")
                for v in range(4):
                    sl = slice(v * 128, (v + 1) * 128)
                    if s2_3m:
                        pa = ps2a.tile([128, H], F32, tag="a2")
                        pb = ps2bc.tile([128, H], F32, tag="b2")
                        for p in range(4):
                            nc.tensor.matmul(pa[:], dr_sb[:, p, sl], xs[:, p, :],
                                             start=(p == 0), stop=(p == 3))
                            nc.tensor.matmul(pb[:], dp_sb[:, p, sl], o1i[:, p, :],
                                             start=(p == 0), stop=(p == 3))
                        pa2_sb = spool.tile([128, H], F32, tag="pas2")
                        nc.scalar.copy(pa2_sb[:], pa[:])
                        fr = spool.tile([128, H], BF16, tag="fr")
                        nc.vector.tensor_sub(fr[:], pa2_sb[:], pb[:])
                        pc = ps2bc.tile([128, H], F32, tag="c2")
                        for p in range(4):
                            nc.tensor.matmul(pc[:], dm_sb[:, p, sl], o1r[:, p, :],
                                             start=(p == 0), stop=(p == 3))
                        fi = spool.tile([128, H], BF16, tag="fi")
                        nc.vector.tensor_add(fi[:], pa2_sb[:], pc[:])
                        nc.scalar.square(prt[:, v, :], fr[:])
                        nc.scalar.square(pit[:, v, :], fi[:])
                    else:
                        pa = ps2a.tile([128, H], F32, tag="a2")
                        pb = ps2a.tile([128, H], F32, tag="b2")
                        for p in range(4):
                            drp = dr_sb[:, p, sl]
                            nc.tensor.matmul(pa[:], drp, o1r[:, p, :],
                                             start=(p == 0), stop=False)
                            nc.tensor.matmul(pb[:], drp, o1i[:, p, :],
                                             start=(p == 0), stop=False)
                        for p in range(4):
                            nc.tensor.matmul(pa[:], dn_sb[:, p, sl], o1i[:, p, :],
                                             start=False, stop=(p == 3))
                            nc.tensor.matmul(pb[:], di_sb[:, p, sl], o1r[:, p, :],
                                             start=False, stop=(p == 3))
                        nc.scalar.square(prt[:, v, :], pa[:])
                        nc.scalar.square(pit[:, v, :], pb[:])
                t = bigsc.tile([128, 4, H], BF16, tag="t")
                te = nc.gpsimd if tadd_eng == "pool" else nc.vector
                te.tensor_add(t[:], prt[:], pit[:])
                gs = bigsc.tile([128, 4, H], BF16, tag="t")
                se = nc.gpsimd if tadd_eng == "pool" else nc.vector
                se.scalar_tensor_tensor(
                    out=gs[:], in0=t[:], scalar=0.0, in1=wt_sb[:],
                    op0=ALU.bypass, op1=ALU.mult,
                    accum_out=acc[:, pr: pr + 1])

            nc.sync.dma_start(out=out.ap(), in_=acc[:])

    nc.compile()
    return nc


def kernel(predictions, ground_truths, band_weights, band_masks):
    global last_results, last_nc, last_in_maps
    pred = np.ascontiguousarray(np.asarray(predictions, dtype=np.float32))
    gt = np.ascontiguousarray(np.asarray(ground_truths, dtype=np.float32))
    bw = np.asarray(band_weights, dtype=np.float64)
    bm = np.asarray(band_masks, dtype=np.float64)

    # host-side prep of tiny replicated constants
    wmap = np.einsum('b,bhw->hw', bw, bm)          # shifted coords
    wu = np.fft.ifftshift(wmap)                     # unshifted coords
    bf = ml_dtypes.bfloat16
    wtb = np.ascontiguousarray(wu.T.astype(bf))
    j = np.arange(H, dtype=np.float64)
    ang = 2.0 * np.pi * np.outer(j, j) / H
    scale = 1.0 / np.sqrt(H)
    drm = np.cos(ang) * scale
    dim = -np.sin(ang) * scale
    drb = np.ascontiguousarray(drm.astype(bf))
    dib = np.ascontiguousarray(dim.astype(bf))
    dpb = np.ascontiguousarray((drm + dim).astype(bf))
    dmb = np.ascontiguousarray((dim - drm).astype(bf))
    dnb = np.ascontiguousarray((-dim).astype(bf))

    pred_r = pred.reshape(N_CORES, IMGS_PER_CORE, H, H)
    gt_r = gt.reshape(N_CORES, IMGS_PER_CORE, H, H)
    in_maps = [
        {
            "pred": np.ascontiguousarray(pred_r[c]),
            "gt": np.ascontiguousarray(gt_r[c]),
            "d_r": drb, "d_i": dib, "d_p": dpb, "d_m": dmb, "d_n": dnb,
            "wt": wtb,
        }
        for c in range(N_CORES)
    ]

    nc = _build_nc()
    last_nc, last_in_maps = nc, in_maps
    res = run_bass_kernel_spmd(nc, in_maps, core_ids=list(range(N_CORES)))
    last_results = res
    total = np.float64(0.0)
    for r in res.results:
        total += r["out"].astype(np.float64).sum()
    loss = total / float(N * C * H * H)
    return np.float32(loss)


# revision 18
# speedup vs baseline: 1.3628x; 1.0045x over previous
"""Trainium2 Bass kernel for DifferentiableWeightedRadialFrequencyLoss.

Math:
  loss = sum_{n,c,u,v} Wmap[u,v] * |FFT2(pred-gt)[u,v]|^2 / size
with Wmap = sum_b w_b * mask_b (bands disjoint), in unshifted (ifftshift)
frequency coordinates.

Device algorithm (per core, 12 images = 6 pairs):
  - pack two real images per complex FFT: Z = E1 + i*E2 (Wmap is symmetric
    under (u,v) -> (-u,-v), so cross terms cancel exactly).
  - FFT2 as two matmul stages with the symmetric ortho DFT matrix D:
      out1 = Z^T @ D      (stage 1)
      F^T  = D @ out1     (stage 2)
    each complex product via 3-multiplication Karatsuba:
      (A+iB)@(C+iD): m1=A@(C+D), m2=(A+B)@D, m3=(B-A)@C
                     real=m1-m2, imag=m1+m3
    with the constant-side combos (Dr+Di, Di-Dr) precomputed on host.
  - weighted power: P = Fr^2 + Fi^2 (ACT squares), acc += row-sum(P .* W^T)
    via DVE scalar_tensor_tensor accum_out.
Host: shard batch across 8 cores, sum partial accumulators, divide by size.
"""

import numpy as np
import ml_dtypes

import concourse.bass as bass
import concourse.bacc as bacc
import concourse.tile as tile
from concourse import mybir
from concourse.bass_utils import run_bass_kernel_spmd

N_CORES = 8
N, C, H = 32, 3, 512
NUM_BANDS = 16
IMGS_PER_CORE = (N // N_CORES) * C          # 12
PAIRS = IMGS_PER_CORE // 2                  # 6
F32 = mybir.dt.float32
BF16 = mybir.dt.bfloat16
ALU = mybir.AluOpType

# exposed for test.py introspection
last_results = None
last_nc = None
last_in_maps = None


def _build_nc(s2_3m=True, combo_eng="dve", xs_chunked=True,
              zbufs=3, o1bufs=2, ldbufs=2, tadd_eng="dve", spbufs=3):
    nc = bacc.Bacc("TRN2", target_bir_lowering=False, debug=False,
                   num_devices=N_CORES)
    pred = nc.dram_tensor("pred", [IMGS_PER_CORE, H, H], F32, kind="ExternalInput")
    gt = nc.dram_tensor("gt", [IMGS_PER_CORE, H, H], F32, kind="ExternalInput")
    d_r = nc.dram_tensor("d_r", [H, H], BF16, kind="ExternalInput")   # Dr
    d_i = nc.dram_tensor("d_i", [H, H], BF16, kind="ExternalInput")   # Di
    d_p = nc.dram_tensor("d_p", [H, H], BF16, kind="ExternalInput")   # Dr+Di
    d_m = nc.dram_tensor("d_m", [H, H], BF16, kind="ExternalInput")   # Di-Dr
    d_n = nc.dram_tensor("d_n", [H, H], BF16, kind="ExternalInput")   # -Di
    wt = nc.dram_tensor("wt", [H, H], BF16, kind="ExternalInput")     # W^T
    out = nc.dram_tensor("out", [128, PAIRS], F32, kind="ExternalOutput")

    def r4(ap):  # [512, 512] dram view -> [128 part, 4 chunks, 512]
        return ap.rearrange("(c p) w -> p c w", p=128)

    with tile.TileContext(nc) as tc:
        with (
            tc.tile_pool(name="consts", bufs=1) as consts,
            tc.tile_pool(name="loads", bufs=ldbufs) as loads,
            tc.tile_pool(name="zpool", bufs=zbufs) as zpool,
            tc.tile_pool(name="o1pool", bufs=o1bufs) as o1pool,
            tc.tile_pool(name="spool", bufs=spbufs) as spool,
            tc.tile_pool(name="bigsc", bufs=2) as bigsc,
            tc.tile_pool(name="ps1a", bufs=2, space="PSUM") as ps1a,
            tc.tile_pool(name="ps1bc", bufs=1, space="PSUM") as ps1bc,
            tc.tile_pool(name="ps2a", bufs=2, space="PSUM") as ps2a,
            tc.tile_pool(name="ps2bc", bufs=1, space="PSUM") as ps2bc,
        ):
            dr_sb = consts.tile([128, 4, H], BF16)
            di_sb = consts.tile([128, 4, H], BF16)
            dp_sb = consts.tile([128, 4, H], BF16)
            dm_sb = consts.tile([128, 4, H], BF16)
            dn_sb = None if s2_3m else consts.tile([128, 4, H], BF16)
            wt_sb = consts.tile([128, 4, H], BF16)
            acc = consts.tile([128, PAIRS], F32)
            nc.sync.dma_start(out=dr_sb[:], in_=r4(d_r.ap()))
            nc.sync.dma_start(out=di_sb[:], in_=r4(d_i.ap()))
            nc.sync.dma_start(out=dp_sb[:], in_=r4(d_p.ap()))
            nc.sync.dma_start(out=dm_sb[:], in_=r4(d_m.ap()))
            if dn_sb is not None:
                nc.sync.dma_start(out=dn_sb[:], in_=r4(d_n.ap()))
            nc.sync.dma_start(out=wt_sb[:], in_=r4(wt.ap()))

            for pr in range(PAIRS):
                i1, i2 = 2 * pr, 2 * pr + 1
                p1t = loads.tile([128, 4, H], F32, tag="p1t")
                g1t = loads.tile([128, 4, H], F32, tag="g1t")
                p2t = loads.tile([128, 4, H], F32, tag="p2t")
                g2t = loads.tile([128, 4, H], F32, tag="g2t")
                nc.sync.dma_start(out=p1t[:], in_=r4(pred.ap()[i1]))
                nc.sync.dma_start(out=g1t[:], in_=r4(gt.ap()[i1]))
                nc.sync.dma_start(out=p2t[:], in_=r4(pred.ap()[i2]))
                nc.sync.dma_start(out=g2t[:], in_=r4(gt.ap()[i2]))

                # data tiles: zr = E1, zi = E2, zs = zr+zi, zd = zi-zr
                zr = zpool.tile([128, 4, H], BF16, tag="zr")
                zi = zpool.tile([128, 4, H], BF16, tag="zi")
                zs = zpool.tile([128, 4, H], BF16, tag="zs")
                zd = zpool.tile([128, 4, H], BF16, tag="zd")
                ce = nc.gpsimd if combo_eng == "pool" else nc.vector
                nc.gpsimd.tensor_sub(zr[:], p1t[:], g1t[:])
                nc.vector.tensor_sub(zi[:], p2t[:], g2t[:])
                ce.tensor_add(zs[:], zr[:], zi[:])
                ce.tensor_sub(zd[:], zi[:], zr[:])

                # stage 1: out1 = Z^T @ D via 3M
                o1r = o1pool.tile([128, 4, H], BF16, tag="o1r")
                o1i = o1pool.tile([128, 4, H], BF16, tag="o1i")
                for m in range(4):
                    sl = slice(m * 128, (m + 1) * 128)
                    pa = ps1a.tile([128, H], F32, tag="a")
                    pb = ps1bc.tile([128, H], F32, tag="b")
                    for k in range(4):
                        nc.tensor.matmul(pa[:], zr[:, k, sl], dp_sb[:, k, :],
                                         start=(k == 0), stop=(k == 3))
                        nc.tensor.matmul(pb[:], zs[:, k, sl], di_sb[:, k, :],
                                         start=(k == 0), stop=(k == 3))
                    pa_sb = spool.tile([128, H], F32, tag="pas")
                    nc.scalar.copy(pa_sb[:], pa[:])
                    nc.vector.tensor_sub(o1r[:, m, :], pa_sb[:], pb[:])
                    pc = ps1bc.tile([128, H], F32, tag="c")
                    for k in range(4):
                        nc.tensor.matmul(pc[:], zd[:, k, sl], dr_sb[:, k, :],
                                         start=(k == 0), stop=(k == 3))
                    nc.vector.tensor_add(o1i[:, m, :], pa_sb[:], pc[:])
                xs = o1pool.tile([128, 4, H], BF16, tag="xs")
                if s2_3m:
                    if xs_chunked:
                        for m in range(4):
                            ce.tensor_add(xs[:, m, :], o1r[:, m, :], o1i[:, m, :])
                    else:
                        ce.tensor_add(xs[:], o1r[:], o1i[:])

                # stage 2: F^T = D @ out1 via 3M ; weighted power accumulate
                prt = bigsc.tile([128, 4, H], BF16, tag="prt")
                pit = bigsc.tile([128, 4, H], BF16, tag="pit")
                for v in range(4):
                    sl = slice(v * 128, (v + 1) * 128)
                    if s2_3m:
                        pa = ps2a.tile([128, H], F32, tag="a2")
                        pb = ps2bc.tile([128, H], F32, tag="b2")
                        for p in range(4):
                            nc.tensor.matmul(pa[:], dr_sb[:, p, sl], xs[:, p, :],
                                             start=(p == 0), stop=(p == 3))
                            nc.tensor.matmul(pb[:], dp_sb[:, p, sl], o1i[:, p, :],
                                             start=(p == 0), stop=(p == 3))
                        pa2_sb = spool.tile([128, H], F32, tag="pas2")
                        nc.scalar.copy(pa2_sb[:], pa[:])
                        fr = spool.tile([128, H], BF16, tag="fr")
                        nc.vector.tensor_sub(fr[:], pa2_sb[:], pb[:])
                        pc = ps2bc.tile([128, H], F32, tag="c2")
                        for p in range(4):
                            nc.tensor.matmul(pc[:], dm_sb[:, p, sl], o1r[:, p, :],
                                             start=(p == 0), stop=(p == 3))
                        fi = spool.tile([128, H], BF16, tag="fi")
                        nc.vector.tensor_add(fi[:], pa2_sb[:], pc[:])
                        nc.scalar.square(prt[:, v, :], fr[:])
                        nc.scalar.square(pit[:, v, :], fi[:])
                    else:
                        pa = ps2a.tile([128, H], F32, tag="a2")
                        pb = ps2a.tile([128, H], F32, tag="b2")
                        for p in range(4):
                            drp = dr_sb[:, p, sl]
                            nc.tensor.matmul(pa[:], drp, o1r[:, p, :],
                                             start=(p == 0), stop=False)
                            nc.tensor.matmul(pb[:], drp, o1i[:, p, :],
                                             start=(p == 0), stop=False)
                        for p in range(4):
                            nc.tensor.matmul(pa[:], dn_sb[:, p, sl], o1i[:, p, :],
                                             start=False, stop=(p == 3))
                            nc.tensor.matmul(pb[:], di_sb[:, p, sl], o1r[:, p, :],
                                             start=False, stop=(p == 3))
                        nc.scalar.square(prt[:, v, :], pa[:])
                        nc.scalar.square(pit[:, v, :], pb[:])
                t = bigsc.tile([128, 4, H], BF16, tag="t")
                te = nc.gpsimd if tadd_eng == "pool" else nc.vector
                te.tensor_add(t[:], prt[:], pit[:])
                gs = bigsc.tile([128, 4, H], BF16, tag="t")
                se = nc.gpsimd if tadd_eng == "pool" else nc.vector
                se.scalar_tensor_tensor(
                    out=gs[:], in0=t[:], scalar=0.0, in1=wt_sb[:],
                    op0=ALU.bypass, op1=ALU.mult,
                    accum_out=acc[:, pr: pr + 1])

            nc.sync.dma_start(out=out.ap(), in_=acc[:])

    nc.compile()
    return nc


def kernel(predictions, ground_truths, band_weights, band_masks):
    global last_results, last_nc, last_in_maps
    pred = np.ascontiguousarray(np.asarray(predictions, dtype=np.float32))
    gt = np.ascontiguousarray(np.asarray(ground_truths, dtype=np.float32))
    bw = np.asarray(band_weights, dtype=np.float64)
    bm = np.asarray(band_masks, dtype=np.float64)

    # host-side prep of tiny replicated constants
    wmap = np.einsum('b,bhw->hw', bw, bm)          # shifted coords
    wu = np.fft.ifftshift(wmap)                     # unshifted coords
    bf = ml_dtypes.bfloat16
    wtb = np.ascontiguousarray(wu.T.astype(bf))
    j = np.arange(H, dtype=np.float64)
    ang = 2.0 * np.pi * np.outer(j, j) / H
    scale = 1.0 / np.sqrt(H)
    drm = np.cos(ang) * scale
    dim = -np.sin(ang) * scale
    drb = np.ascontiguousarray(drm.astype(bf))
    dib = np.ascontiguousarray(dim.astype(bf))
    dpb = np.ascontiguousarray((drm + dim).astype(bf))
    dmb = np.ascontiguousarray((dim - drm).astype(bf))
    dnb = np.ascontiguousarray((-dim).astype(bf))

    pred_r = pred.reshape(N_CORES, IMGS_PER_CORE, H, H)
    gt_r = gt.reshape(N_CORES, IMGS_PER_CORE, H, H)
    in_maps = [
        {
            "pred": np.ascontiguousarray(pred_r[c]),
            "gt": np.ascontiguousarray(gt_r[c]),
            "d_r": drb, "d_i": dib, "d_p": dpb, "d_m": dmb, "d_n": dnb,
            "wt": wtb,
        }
        for c in range(N_CORES)
    ]

    nc = _build_nc()
    last_nc, last_in_maps = nc, in_maps
    res = run_bass_kernel_spmd(nc, in_maps, core_ids=list(range(N_CORES)))
    last_results = res
    total = np.float64(0.0)
    for r in res.results:
        total += r["out"].astype(np.float64).sum()
    loss = total / float(N * C * H * H)
    return np.float32(loss)
